# revision 42
# baseline (speedup 1.0000x reference)
"""Tensor-parallel causal attention (GQA, rotary) for Trainium2, 8 NeuronCores.

Problem: x[2,2048,2048] -> QKV proj -> rotary -> 32-head causal attention
(8 kv heads, head_dim 64) -> out @ wo, fp32 reference.

Sharding (tensor-parallel over heads): core c owns q heads [4c,4c+4) and kv
head c. Each core computes its heads' attention and a partial output
projection partial_c = attn_c @ wo[256c:256c+256]; the host sums 8 partials.

Matmuls run in bf16 (QKV, scores, PV) and float32r (projection), with fp32
psum accumulation throughout; end-to-end relative error vs the fp32
reference is ~5e-3 (gate 2e-2).

Per-core pipeline (phases overlap via the Tile scheduler):
  A) Fused QKV projection from host-pre-transposed x (xT [D, B*S] streamed
     as tiles on two HWDGE queues; block 0 streams per-128-chunk for a fast
     start). Weights stationary [128-chunk, 384]; psum ring of 6 so rotary
     consumption never blocks the next block's matmuls. Host permutes W
     columns so rotary even/odd pairs arrive deinterleaved. Rotary (DVE)
     reads psum directly; Q lands in QF pair tiles, K is replicated to rows
     64:128 (SBUF-SBUF DMA on the gpsimd SWDGE queue) for 2-head row
     packing; V.T is PE-transposed into (V|1) tiles (ones column pre-set at
     init -> PV emits softmax denominators).
  C) Scores transposed, S.T[k,q] = K-tile.T @ Q.T, two heads row-packed per
     pass. Causal masking folds into the score psum accumulation: a second
     matmul (bf16 identity stationary x precomputed -1e30 mask tile) adds
     the mask before exp, so no gpsimd op sits on the critical path. exp on
     ACT (scale=1/8) over live column ranges only. PV accumulates
     (V|1).T @ P.T into a [65,1024] psum (heads A|B side by side); row 64 is
     the denominator. Normalize chain (all off the PE): DVE evicts pv to
     SBUF (frees the psum bank), ACT computes 1/den = exp(-ln(den)) (ln and
     exp share one ACT table set), gpsimd partition_broadcast spreads the
     reciprocal row to 64 partitions, DVE multiplies into ATT.
  D) Output projection interleaved INTO the next (b,j) attention t-loop as
     8 drip-fed work units per block, keeping the PE busy while ACT runs
     exp: OUT row-tile mt x 512-col block, psum [128,512] x2 bufs, evictions
     alternate DVE/ACT, DMA on both HWDGE queues.

PSUM budget (8 banks): A: qkv ring 6 + V-transpose 2; C/D: scores 2x2 +
PV [65,1024] 2 + proj 2x1.
"""
import numpy as np

B, S, D = 2, 2048, 2048
H, KV, HD = 32, 8, 64
NCORES = 8
HPC = H // NCORES          # 4 q heads per core
TOKS = B * S               # 4096
DCH = D // 128             # 16 contraction chunks
NBLK = TOKS // 512         # 8 token blocks of 512
QB = 512                   # q block size (phase C)
KTILES = S // 128          # 16 k tiles per batch

_CACHE = {}
QKV_BF16 = True        # stream xT/W in bf16 (halves phase-A HBM traffic)
OUT_BF16 = True        # write OUT partials in bf16 (halves write traffic)


def _build(reps=1, qkv_bf16=None, debug=False, phases="acd", out_bf16=None):
    """reps>1 statically unrolls the whole pipeline for timing runs
    (dispatch overhead cancels in the difference between reps values)."""
    import concourse.bacc as bacc
    import concourse.mybir as mybir
    from concourse import tile

    # The ACT-table chooser picks the FIRST set containing each function:
    # Exp -> exp_and_others, Ln -> natural_log, neither serves the other,
    # so every normalize thrashes ACT_TABLE_LOAD (~1.3us each). Empty those
    # two sets (keeping dict order, so act_func_set_ids stay canonical) and
    # both functions resolve to natural_log_exp_and_others: one load total.
    if not getattr(bacc, "_act_tables_patched", False):
        _orig_gat = bacc.get_activation_tables

        def _patched_gat(arch):
            t = dict(_orig_gat(arch))
            t["exp_and_others"] = set()
            t["natural_log"] = set()
            return t

        bacc.get_activation_tables = _patched_gat
        bacc._act_tables_patched = True

    if qkv_bf16 is None:
        qkv_bf16 = QKV_BF16
    if out_bf16 is None:
        out_bf16 = OUT_BF16
    F32 = mybir.dt.float32
    F32R = mybir.dt.float32r
    BF16 = mybir.dt.bfloat16
    XDT = BF16 if qkv_bf16 else F32R
    ODT = BF16 if out_bf16 else F32
    EXP = mybir.ActivationFunctionType.Exp
    LN = mybir.ActivationFunctionType.Ln

    nc = bacc.Bacc()
    xT = nc.declare_dram_parameter("xT", [D, TOKS], XDT, isOutput=False)
    W = nc.declare_dram_parameter("W", [D, 384], XDT, isOutput=False)
    WO = nc.declare_dram_parameter("WO", [256, D], F32R, isOutput=False)
    CS = nc.declare_dram_parameter("CS", [128, S], F32, isOutput=False)
    SN = nc.declare_dram_parameter("SN", [128, S], F32, isOutput=False)
    MSK = nc.declare_dram_parameter("MSK", [128, 2048], BF16, isOutput=False)
    IDN = nc.declare_dram_parameter("IDN", [128, 128], F32, isOutput=False)
    OUT = nc.declare_dram_parameter("OUT", [TOKS, D], ODT, isOutput=True)
    if debug:
        QF_d = nc.declare_dram_parameter("QF_d", [128, 2 * TOKS], BF16, isOutput=True)
        KF_d = nc.declare_dram_parameter("KF_d", [128, TOKS], BF16, isOutput=True)
        VH_d = nc.declare_dram_parameter("VH_d", [128, 2 * KTILES * 65], BF16, isOutput=True)
        ATT_d = nc.declare_dram_parameter("ATT_d", [128, 2 * TOKS], F32, isOutput=True)

    with tile.TileContext(nc) as tc:
        with tc.tile_pool(name="const", bufs=1) as cp:
            # -- startup-critical DMA ordering (per-queue FIFO):
            #    sync:   W even chunks, then block-0 x quarters (even)
            #    scalar: CS/SN chunk 0, W odd chunks, block-0 x quarters (odd)
            #    gpsimd (SWDGE): WO, MSK, IDN (not needed until later)
            W_sb = cp.tile([128, DCH * 384], XDT)        # 24KB/part (12KB bf16)
            CS_sb = cp.tile([128, S], F32)
            SN_sb = cp.tile([128, S], F32)
            # one big W gather (per-DMA completion overhead dominates small
            # startup transfers); CS/SN full tiles on the other queue
            nc.sync.dma_start(out=W_sb[:],
                              in_=W.rearrange("(k p) c -> p k c", p=128))
            nc.scalar.dma_start(out=CS_sb[:], in_=CS[:])
            nc.scalar.dma_start(out=SN_sb[:], in_=SN[:])
            WO_sb = cp.tile([128, 2 * D], F32R)          # 16KB/part
            for f in range(2):
                nc.gpsimd.dma_start(out=WO_sb[:, f * D:(f + 1) * D],
                                    in_=WO[f * 128:(f + 1) * 128, :])
            MSK_sb = cp.tile([128, 2048], BF16)
            nc.gpsimd.dma_start(out=MSK_sb[:], in_=MSK[:])
            ident = cp.tile([128, 128], F32)
            nc.gpsimd.dma_start(out=ident[:], in_=IDN[:])
            identb = cp.tile([128, 128], BF16)
            nc.vector.tensor_copy(identb[:], ident[:])
            QF = cp.tile([128, 2 * TOKS], BF16)          # 16KB: pairtile p at cols p*TOKS
            KF = cp.tile([128, TOKS], BF16)              # 8KB: rows 0:64 K, 64:128 replica
            VH = cp.tile([128, 2 * KTILES * 65], BF16)   # 4.2KB: (b*16+t)*65 | V,1
            ATT = cp.tile([128, 2 * TOKS], F32R)         # 32KB: ftile f at cols f*TOKS
            ones_f = cp.tile([128, 1], F32)
            nc.vector.memset(ones_f[:], 1.0)
            for idx in range(2 * KTILES):                # (V|1) ones columns, once
                nc.vector.tensor_copy(VH[:, idx * 65 + 64: idx * 65 + 65],
                                      ones_f[:])

            def _emit_body():
              # ---------------- Phase A: QKV projection + rotary + V transpose
              with (
                  tc.tile_pool(name="xa", bufs=7) as xap,
                  tc.tile_pool(name="pa", bufs=1, space="PSUM") as pap,
                  tc.tile_pool(name="ta", bufs=2) as tap,
              ):
                  for n in range(NBLK):
                      b = n // 4
                      ccols = slice((n % 4) * 512, (n % 4) * 512 + 512)
                      ncols = slice(n * 512, (n + 1) * 512)
                      pss = [pap.tile([128, 512], F32, tag="qkv", bufs=6,
                                      name=f"ps_{n}_{m}") for m in range(3)]
                      xr = xT.rearrange("(k p) t -> p k t", p=128)
                      qtr = DCH // 4
                      xhs = []
                      for hh in range(4):
                          xh = xap.tile([128, qtr * 512], XDT, tag="xt",
                                        name=f"xt_{n}_{hh}")
                          if n == 0:
                              # per-chunk quarters so matmul k can start as
                              # soon as its chunk lands
                              for q in range(qtr):
                                  k = hh * qtr + q
                                  xeng = nc.sync if k % 2 == 0 else nc.scalar
                                  xeng.dma_start(
                                      out=xh[:, q * 512:(q + 1) * 512],
                                      in_=xr[:, k, ncols])
                          else:
                              xeng = nc.sync if hh % 2 == 0 else nc.scalar
                              xeng.dma_start(out=xh[:],
                                             in_=xr[:, hh * qtr:(hh + 1) * qtr, ncols])
                          xhs.append(xh)
                      for k in range(DCH):
                          xt = xhs[k // qtr][:, (k % qtr) * 512:(k % qtr + 1) * 512]
                          for m in range(3):
                              nc.tensor.matmul(
                                  pss[m][:],
                                  W_sb[:, k * 384 + m * 128: k * 384 + (m + 1) * 128],
                                  xt, start=(k == 0), stop=(k == DCH - 1))
                      # rotary Q: TR=pss[0], TI=pss[1] (even/odd deinterleaved).
                      # Products land in bf16 so the combines below hit DVE's
                      # 2x packed mode (both inputs 16-bit, unit stride).
                      t1 = tap.tile([128, 512], BF16, tag="t1", name=f"t1_{n}")
                      t2 = tap.tile([128, 512], BF16, tag="t2", name=f"t2_{n}")
                      nc.vector.tensor_mul(t1[:], pss[0][:], CS_sb[:, ccols])
                      nc.vector.tensor_mul(t2[:], pss[1][:], SN_sb[:, ccols])
                      for h in range(HPC):
                          base = (h // 2) * TOKS + n * 512
                          nc.vector.tensor_sub(
                              QF[64 * (h % 2): 64 * (h % 2) + 32, base: base + 512],
                              t1[32 * h:32 * h + 32, :], t2[32 * h:32 * h + 32, :])
                      t3 = tap.tile([128, 512], BF16, tag="t1", name=f"t3_{n}")
                      t4 = tap.tile([128, 512], BF16, tag="t2", name=f"t4_{n}")
                      nc.vector.tensor_mul(t3[:], pss[0][:], SN_sb[:, ccols])
                      nc.vector.tensor_mul(t4[:], pss[1][:], CS_sb[:, ccols])
                      for h in range(HPC):
                          base = (h // 2) * TOKS + n * 512
                          nc.vector.tensor_add(
                              QF[64 * (h % 2) + 32: 64 * (h % 2) + 64, base: base + 512],
                              t3[32 * h:32 * h + 32, :], t4[32 * h:32 * h + 32, :])
                      # rotary K: rows 0:32 even, 32:64 odd of pss[2]
                      tk1 = tap.tile([32, 512], BF16, tag="t1", name=f"tk1_{n}")
                      tk2 = tap.tile([32, 512], BF16, tag="t2", name=f"tk2_{n}")
                      nc.vector.tensor_mul(tk1[:], pss[2][0:32, :], CS_sb[0:32, ccols])
                      nc.vector.tensor_mul(tk2[:], pss[2][32:64, :], SN_sb[32:64, ccols])
                      nc.vector.tensor_sub(KF[0:32, ncols], tk1[:], tk2[:])
                      tk3 = tap.tile([32, 512], BF16, tag="t1", name=f"tk3_{n}")
                      tk4 = tap.tile([32, 512], BF16, tag="t2", name=f"tk4_{n}")
                      nc.vector.tensor_mul(tk3[:], pss[2][0:32, :], SN_sb[0:32, ccols])
                      nc.vector.tensor_mul(tk4[:], pss[2][32:64, :], CS_sb[32:64, ccols])
                      nc.vector.tensor_add(KF[32:64, ncols], tk3[:], tk4[:])
                      # replicate this block's K rows for 2-head row packing
                      nc.gpsimd.dma_start(out=KF[64:128, ncols], in_=KF[0:64, ncols])
                      # V: evict rows 64:128, PE-transpose 128-tok chunks into VH
                      vs = tap.tile([64, 512], BF16, tag="t2", name=f"vs_{n}")
                      nc.scalar.copy(vs[:], pss[2][64:128, :])
                      for q in range(4):
                          t_global = (n % 4) * 4 + q      # ktile within batch
                          idx = b * KTILES + t_global
                          vt_ps = pap.tile([128, 64], BF16, tag="vt", bufs=2, name=f"vt_{n}_{q}")
                          nc.tensor.transpose(vt_ps[:], vs[:, q * 128:(q + 1) * 128],
                                              identb[0:64, 0:64])
                          nc.scalar.copy(VH[:, idx * 65: idx * 65 + 64], vt_ps[:])


              if "c" not in phases:
                  return
              # ------- Phase C+D fused: attention; prior block's projection
              # work units drip-fed into the t-loop to keep the PE busy.
              with (
                  tc.tile_pool(name="sc", bufs=2, space="PSUM") as scp,
                  tc.tile_pool(name="pv", bufs=1, space="PSUM") as pvp,
                  tc.tile_pool(name="pd", bufs=1, space="PSUM") as pdp,
                  tc.tile_pool(name="pt", bufs=6) as ptp,
                  tc.tile_pool(name="nm", bufs=2) as nmp,
                  tc.tile_pool(name="od", bufs=6) as odp,
              ):
                  def proj_unit(mt, cbp):
                      # OUT rows mt*128, cols cbp*1024: two [128,512] psums,
                      # each accumulating f=0,1; one LDW per (f, pair)
                      pss_ = [pdp.tile([128, 512], F32, tag="pd", bufs=2,
                                       name=f"pd_{mt}_{cbp}_{z}") for z in range(2)]
                      for f in range(2):
                          st = ATT[:, f * TOKS + mt * 128: f * TOKS + (mt + 1) * 128]
                          for z in range(2):
                              col = f * D + (2 * cbp + z) * 512
                              nc.tensor.matmul(pss_[z][:], st,
                                               WO_sb[:, col: col + 512],
                                               start=(f == 0), stop=(f == 1))
                      for z in range(2):
                          cb = 2 * cbp + z
                          os_ = odp.tile([128, 512], ODT, tag="od",
                                         name=f"od_{mt}_{cb}")
                          if (mt + cb) % 4 != 0:     # ACT is busier: bias DVE
                              nc.vector.tensor_copy(os_[:], pss_[z][:])
                          else:
                              nc.scalar.copy(os_[:], pss_[z][:])
                          eng = nc.sync if (mt + cb) % 2 == 0 else nc.scalar
                          eng.dma_start(
                              out=OUT[mt * 128:(mt + 1) * 128,
                                      cb * 512:(cb + 1) * 512],
                              in_=os_[:])

                  proj_q = []          # queued (mt, cbp) units from prior block

                  def drip(k=1):
                      for _ in range(min(k, len(proj_q))):
                          proj_unit(*proj_q.pop(0))

                  for b in range(B):
                      for j in range(4):          # q block of 512 within batch
                          for pr in range(2):     # head pair
                              pv = pvp.tile([65, 1024], F32, tag="pv",
                                            name=f"pv_{b}_{pr}_{j}")
                              nk = 4 * (j + 1)
                              for t in range(nk):
                                  kc = slice(b * S + t * 128, b * S + (t + 1) * 128)
                                  i = t - 4 * j
                                  # bf16 matmuls run 1 cyc/row at any N, so
                                  # compute exactly the live columns
                                  lo = 0 if i < 0 else 128 * i
                                  sc = scp.tile([128, 1024], F32, tag="sc",
                                                name=f"sc_{b}_{pr}_{j}_{t}")
                                  q0 = pr * TOKS + b * S + j * 512
                                  nc.tensor.matmul(
                                      sc[:, lo:512], KF[0:64, kc],
                                      QF[0:64, q0 + lo: q0 + 512],
                                      start=True, stop=(i < 0))
                                  nc.tensor.matmul(
                                      sc[:, 512 + lo:1024], KF[64:128, kc],
                                      QF[64:128, q0 + lo: q0 + 512],
                                      start=True, stop=(i < 0))
                                  if i >= 0:
                                      # fold causal mask into the psum accum:
                                      # += I.T @ MSK_i (0 live / -1e30 masked)
                                      mski = MSK_sb[:, i * 512 + lo: (i + 1) * 512]
                                      nc.tensor.matmul(
                                          sc[:, lo:512], identb[:],
                                          mski, start=False, stop=True)
                                      nc.tensor.matmul(
                                          sc[:, 512 + lo:1024], identb[:],
                                          mski, start=False, stop=True)
                                  pt = ptp.tile([128, 1024], BF16, tag="pt",
                                                name=f"pt_{b}_{pr}_{j}_{t}")
                                  if lo == 0:
                                      nc.scalar.activation(pt[:], sc[:], EXP,
                                                           scale=0.125)
                                  else:
                                      nc.scalar.activation(pt[:, lo:512],
                                                           sc[:, lo:512],
                                                           EXP, scale=0.125)
                                      nc.scalar.activation(pt[:, 512 + lo:1024],
                                                           sc[:, 512 + lo:1024],
                                                           EXP, scale=0.125)
                                  vcol = VH[:, (b * KTILES + t) * 65:
                                            (b * KTILES + t) * 65 + 65]
                                  nc.tensor.matmul(pv[:, lo:512], vcol,
                                                   pt[:, lo:512],
                                                   start=(t == 0), stop=(t == nk - 1))
                                  nc.tensor.matmul(pv[:, 512 + lo:1024], vcol,
                                                   pt[:, 512 + lo:1024],
                                                   start=(t == 0), stop=(t == nk - 1))
                                  drip()
                              # normalize (PE-free): evict psum, 1/den =
                              # exp(-ln(den)) on ACT, partition_broadcast on
                              # gpsimd, multiply into ATT on DVE
                              pv_sb = nmp.tile([65, 1024], F32R, tag="pvsb",
                                               name=f"pvsb_{b}_{pr}_{j}")
                              nc.vector.tensor_copy(pv_sb[:], pv[:])
                              lnr = nmp.tile([1, 1024], F32, tag="lnr",
                                             name=f"lnr_{b}_{pr}_{j}")
                              nc.scalar.activation(lnr[:], pv_sb[64:65, :], LN)
                              rcp = nmp.tile([1, 1024], F32, tag="rcp",
                                             name=f"rcp_{b}_{pr}_{j}")
                              nc.scalar.activation(rcp[:], lnr[:], EXP, scale=-1.0)
                              bc = nmp.tile([64, 1024], F32, tag="bc", bufs=1,
                                            name=f"bc_{b}_{pr}_{j}")
                              nc.gpsimd.partition_broadcast(bc[:], rcp[0:1, :])
                              for hh in range(2):
                                  h = 2 * pr + hh
                                  dst = ATT[64 * (h % 2): 64 * (h % 2) + 64,
                                            (h // 2) * TOKS + b * S + j * 512:
                                            (h // 2) * TOKS + b * S + (j + 1) * 512]
                                  nc.vector.tensor_mul(
                                      dst, pv_sb[0:64, hh * 512:(hh + 1) * 512],
                                      bc[:, hh * 512:(hh + 1) * 512])
                              drip()
                          # queue this block's projection; it runs inside the
                          # NEXT block's t-loop (ATT ready by then)
                          for mq in range(4):
                              mt = b * 16 + j * 4 + mq
                              for cbp in range(2):
                                  proj_q.append((mt, cbp))
                          if "d" not in phases:
                              proj_q.clear()
                  drip(len(proj_q))   # tail block

            for _ in range(reps):
                _emit_body()
            if debug:
                nc.sync.dma_start(out=QF_d[:], in_=QF[:])
                nc.sync.dma_start(out=KF_d[:], in_=KF[:])
                nc.sync.dma_start(out=VH_d[:], in_=VH[:])
                nc.sync.dma_start(out=ATT_d[:], in_=ATT[:].bitcast(F32))

    nc.compile()
    return nc


def _prep_inputs(x, freqs_cos, freqs_sin, wq, wk, wv, wo):
    """Host-side shard prep. Returns per-core input dicts."""
    x = np.asarray(x, dtype=np.float32)
    fc = np.asarray(freqs_cos, dtype=np.float32)
    fs = np.asarray(freqs_sin, dtype=np.float32)
    wq = np.asarray(wq, dtype=np.float32)
    wk = np.asarray(wk, dtype=np.float32)
    wv = np.asarray(wv, dtype=np.float32)
    wo = np.asarray(wo, dtype=np.float32)

    import ml_dtypes
    xT = np.ascontiguousarray(x.transpose(2, 0, 1).reshape(D, TOKS))
    if QKV_BF16:
        xT = xT.astype(ml_dtypes.bfloat16)
    CSa = np.ascontiguousarray(np.tile(fc.T, (4, 1)))   # [128, S]
    SNa = np.ascontiguousarray(np.tile(fs.T, (4, 1)))
    kp = np.arange(128)[:, None]
    qf = np.arange(512)[None, :]
    msk = np.zeros((128, 4 * 512), np.float32)
    for i in range(4):
        msk[:, i * 512:(i + 1) * 512] = np.where(qf >= kp + 128 * i, 0.0, -1e30)
    msk = msk.astype(ml_dtypes.bfloat16)
    idn = np.eye(128, dtype=np.float32)

    in_maps = []
    for c in range(NCORES):
        cols = np.empty(384, dtype=np.int64)
        for h in range(HPC):
            for p in range(32):
                cols[32 * h + p] = (HPC * c + h) * HD + 2 * p          # Q even
                cols[128 + 32 * h + p] = (HPC * c + h) * HD + 2 * p + 1  # Q odd
        qW = wq[:, cols[:256]]
        kcols = np.empty(64, dtype=np.int64)
        kcols[:32] = HD * c + 2 * np.arange(32)
        kcols[32:] = HD * c + 2 * np.arange(32) + 1
        kW = wk[:, kcols]
        vW = wv[:, HD * c: HD * (c + 1)]
        Wc = np.ascontiguousarray(np.concatenate([qW, kW, vW], axis=1))
        if QKV_BF16:
            Wc = Wc.astype(ml_dtypes.bfloat16)
        WOc = np.ascontiguousarray(wo[256 * c: 256 * (c + 1), :])
        in_maps.append({"xT": xT, "W": Wc, "WO": WOc, "CS": CSa, "SN": SNa,
                        "MSK": msk, "IDN": idn})
    return in_maps


def kernel(x, freqs_cos, freqs_sin, wq, wk, wv, wo):
    from concourse.bass_utils import run_bass_kernel_spmd

    if "nc" not in _CACHE:
        _CACHE["nc"] = _build()
    nc = _CACHE["nc"]
    in_maps = _prep_inputs(x, freqs_cos, freqs_sin, wq, wk, wv, wo)
    res = run_bass_kernel_spmd(nc, in_maps, list(range(NCORES)))
    out = np.zeros((TOKS, D), dtype=np.float32)
    for c in range(NCORES):
        out += np.asarray(res.results[c]["OUT"]).astype(np.float32)
    return out.reshape(B, S, D)
